# revision 1
# baseline (speedup 1.0000x reference)
"""Expert-parallel MoE (top-2 of 16 experts, SwiGLU FFN) on 8 TRN2 NeuronCores.

Contract: kernel(**inputs) takes the FULL unsharded inputs and returns the
FULL output (output tensor, load_balancing_loss), matching reference().

Strategy (expert-parallel, per the sharding hint):
  - Host: router matmul + top-2 + softmax (0.3% of FLOPs), then dispatch:
    each core owns 2 experts; its routed tokens are gathered into a packed,
    transposed, zero-padded [D, cap] activation matrix (the "all-to-all").
  - Device (SPMD x8): per expert  yT = W2^T @ (silu(W1^T xT) * (W3^T xT))
    as bf16 matmuls accumulating in f32 PSUM; weights stationary [128,128],
    tokens moving in 512-col blocks.
  - Host: combine: out[tok] += gate * yT[:, col].T  (gate folded here, so the
    device computes the unscaled expert FFN), plus the load-balancing loss.
"""

import os
import sys
import types
from contextlib import ExitStack

import ml_dtypes
import numpy as np

HIDDEN = 1024
FFN = 1024
N_EXPERTS = 16
TOP_K = 2
N_CORES = 8
P = 128

BF16 = ml_dtypes.bfloat16

_KERNEL_CACHE: dict = {}
LAST_RESULTS = None  # BassKernelResults of the most recent device run


# ---------------------------------------------------------------------------
# Optional NTFF profiling support: concourse's trace path imports
# antenv.axon_hooks, which some images lack. Register a compatible module so
# BASS_TRACE=1 profiles when the axon .so supports it and degrades otherwise.
def _ensure_axon_hooks():
    try:
        import antenv.axon_hooks  # noqa: F401
        return
    except ImportError:
        pass
    m = types.ModuleType("antenv.axon_hooks")
    m._hook = None

    def set_axon_ntff_profile_hook(h):
        m._hook = h

    def get_axon_ntff_profile_hook():
        return m._hook

    m.set_axon_ntff_profile_hook = set_axon_ntff_profile_hook
    m.get_axon_ntff_profile_hook = get_axon_ntff_profile_hook
    sys.modules["antenv.axon_hooks"] = m
    try:
        from trn_agent_boot.trn_boot import _ntff_profile_via_ctypes

        so = "/opt/axon/libaxon_pjrt.so"
        if os.path.exists(so):
            m._hook = _ntff_profile_via_ctypes(so)
    except Exception:
        pass


# ---------------------------------------------------------------------------
# Device kernel builder


def _token_blocks(cap):
    blocks = [512] * (cap // 512)
    if cap % 512:
        blocks.append(cap % 512)
    return blocks


def _build_moe_kernel(caps, use_silu=True):
    import concourse.bass as bass
    import concourse.tile as tile
    from concourse import bacc, mybir

    DT = mybir.dt.bfloat16
    PS = mybir.dt.float32
    d, f = HIDDEN, FFN
    KC = d // P
    MC = f // P
    CT = sum(caps)

    nc = bacc.Bacc("TRN2", target_bir_lowering=False, debug=False,
                   num_devices=N_CORES)

    xt = nc.dram_tensor("xt", [d, CT], DT, kind="ExternalInput")
    w1s = nc.dram_tensor("w1s", [2, d, f], DT, kind="ExternalInput")
    w2s = nc.dram_tensor("w2s", [2, f, d], DT, kind="ExternalInput")
    w3s = nc.dram_tensor("w3s", [2, d, f], DT, kind="ExternalInput")
    yt = nc.dram_tensor("yt", [d, CT], PS, kind="ExternalOutput")

    with tile.TileContext(nc) as tc, ExitStack() as ctx:
        wp = ctx.enter_context(tc.tile_pool(name="w", bufs=2))
        xp = ctx.enter_context(tc.tile_pool(name="x", bufs=3))
        hp = ctx.enter_context(tc.tile_pool(name="h", bufs=2))
        yp = ctx.enter_context(tc.tile_pool(name="y", bufs=2))
        pp1 = ctx.enter_context(
            tc.tile_pool(name="ps1", bufs=2, space=bass.MemorySpace.PSUM))
        pp3 = ctx.enter_context(
            tc.tile_pool(name="ps3", bufs=2, space=bass.MemorySpace.PSUM))
        ppy = ctx.enter_context(
            tc.tile_pool(name="psy", bufs=2, space=bass.MemorySpace.PSUM))

        col = 0
        for s in range(2):
            w1t, w3t, w2t = [], [], []
            for k in range(KC):
                t = wp.tile([P, f], DT, tag=f"w1_{k}")
                nc.sync.dma_start(t[:], w1s[s, k * P:(k + 1) * P, :])
                w1t.append(t)
            for k in range(KC):
                t = wp.tile([P, f], DT, tag=f"w3_{k}")
                nc.sync.dma_start(t[:], w3s[s, k * P:(k + 1) * P, :])
                w3t.append(t)
            for m in range(MC):
                t = wp.tile([P, d], DT, tag=f"w2_{m}")
                nc.sync.dma_start(t[:], w2s[s, m * P:(m + 1) * P, :])
                w2t.append(t)

            for n in _token_blocks(caps[s]):
                xts = []
                for k in range(KC):
                    t = xp.tile([P, n], DT, tag=f"x{k}")
                    nc.sync.dma_start(t[:], xt[k * P:(k + 1) * P, col:col + n])
                    xts.append(t)
                hts = []
                for m in range(MC):
                    p1 = pp1.tile([P, n], PS, tag="p1")
                    p3 = pp3.tile([P, n], PS, tag="p3")
                    for k in range(KC):
                        nc.tensor.matmul(
                            p1[:], w1t[k][:, m * P:(m + 1) * P], xts[k][:],
                            start=(k == 0), stop=(k == KC - 1))
                    for k in range(KC):
                        nc.tensor.matmul(
                            p3[:], w3t[k][:, m * P:(m + 1) * P], xts[k][:],
                            start=(k == 0), stop=(k == KC - 1))
                    ht = hp.tile([P, n], DT, tag=f"h{m}")
                    if use_silu:
                        nc.scalar.activation(
                            ht[:], p1[:], mybir.ActivationFunctionType.Silu)
                    else:
                        nc.scalar.activation(
                            ht[:], p1[:], mybir.ActivationFunctionType.Sigmoid)
                        nc.vector.tensor_mul(ht[:], ht[:], p1[:])
                    nc.vector.tensor_mul(ht[:], ht[:], p3[:])
                    hts.append(ht)
                for dd in range(KC):
                    py = ppy.tile([P, n], PS, tag="py")
                    for m in range(MC):
                        nc.tensor.matmul(
                            py[:], w2t[m][:, dd * P:(dd + 1) * P], hts[m][:],
                            start=(m == 0), stop=(m == MC - 1))
                    yo = yp.tile([P, n], PS, tag=f"y{dd}")
                    nc.vector.tensor_copy(yo[:], py[:])
                    nc.sync.dma_start(yt[dd * P:(dd + 1) * P, col:col + n],
                                      yo[:])
                col += n

    nc.compile()
    return nc


def _get_kernel(caps):
    key = tuple(caps)
    if key not in _KERNEL_CACHE:
        _KERNEL_CACHE[key] = _build_moe_kernel(caps)
    return _KERNEL_CACHE[key]


# ---------------------------------------------------------------------------
# Host-side routing / dispatch / combine


def _softmax(x, axis):
    m = np.max(x, axis=axis, keepdims=True)
    e = np.exp(x - m)
    return e / np.sum(e, axis=axis, keepdims=True)


def _roundup(v, q):
    return max(q, ((int(v) + q - 1) // q) * q)


def kernel(hidden_states, router_w, w1, w2, w3):
    global LAST_RESULTS
    _ensure_axon_hooks()
    from concourse.bass_utils import run_bass_kernel_spmd

    hidden_states = np.asarray(hidden_states, dtype=np.float32)
    router_w = np.asarray(router_w, dtype=np.float32)
    w1 = np.asarray(w1, dtype=np.float32)
    w2 = np.asarray(w2, dtype=np.float32)
    w3 = np.asarray(w3, dtype=np.float32)

    B, S, D = hidden_states.shape
    T = B * S
    E = router_w.shape[1]
    x2d = hidden_states.reshape(T, D)

    # --- routing (host) ---
    logits = x2d @ router_w                                   # [T, E] f32
    order = np.argsort(-logits, axis=1, kind="stable")        # lax.top_k ties
    top_idx = order[:, :TOP_K]                                # [T, K]
    top_vals = np.take_along_axis(logits, top_idx, axis=1)
    gates = _softmax(top_vals, axis=1).astype(np.float32)     # [T, K]

    probs = _softmax(logits, axis=1)
    usage = probs.mean(axis=0, dtype=np.float32)
    loss = np.float32(E) * np.sum(usage * usage, dtype=np.float32)
    loss = np.float32(loss)

    # --- dispatch: group (token, gate) pairs by expert ---
    flat_expert = top_idx.ravel()
    flat_tok = np.repeat(np.arange(T, dtype=np.int64), TOP_K)
    flat_gate = gates.ravel()
    grp = np.argsort(flat_expert, kind="stable")
    counts = np.bincount(flat_expert, minlength=E)
    starts = np.zeros(E + 1, np.int64)
    np.cumsum(counts, out=starts[1:])
    tok_of = [flat_tok[grp[starts[e]:starts[e + 1]]] for e in range(E)]
    gate_of = [flat_gate[grp[starts[e]:starts[e + 1]]] for e in range(E)]

    # assign experts to (core, slot): slot0 = 8 busiest, slot1 = 8 smallest,
    # paired big-with-small so per-slot capacity (a max over cores) is tight
    by_load = np.argsort(-counts, kind="stable")
    slot_experts = [[int(by_load[c]), int(by_load[15 - c])]
                    for c in range(N_CORES)]
    cap0 = _roundup(max(counts[es[0]] for es in slot_experts), 256)
    cap1 = _roundup(max(counts[es[1]] for es in slot_experts), 256)
    caps = (cap0, cap1)
    CT = cap0 + cap1

    xbf = x2d.astype(BF16)
    w1bf = w1.astype(BF16)
    w2bf = w2.astype(BF16)
    w3bf = w3.astype(BF16)

    in_maps = []
    for c in range(N_CORES):
        e0, e1 = slot_experts[c]
        xt = np.zeros((D, CT), BF16)
        xt[:, :len(tok_of[e0])] = xbf[tok_of[e0]].T
        xt[:, cap0:cap0 + len(tok_of[e1])] = xbf[tok_of[e1]].T
        in_maps.append({
            "xt": xt,
            "w1s": np.stack([w1bf[e0], w1bf[e1]]),
            "w2s": np.stack([w2bf[e0], w2bf[e1]]),
            "w3s": np.stack([w3bf[e0], w3bf[e1]]),
        })

    nc = _get_kernel(caps)
    res = run_bass_kernel_spmd(nc, in_maps, list(range(N_CORES)))
    LAST_RESULTS = res

    # --- combine (host): out[tok] += gate * yT[:, col]  ---
    out2d = np.zeros((T, D), np.float32)
    for c in range(N_CORES):
        ytc = res.results[c]["yt"]                            # [D, CT] f32
        for s, off in ((0, 0), (1, cap0)):
            e = slot_experts[c][s]
            n = len(tok_of[e])
            if n == 0:
                continue
            contrib = ytc[:, off:off + n].T                   # [n, D]
            out2d[tok_of[e]] += gate_of[e][:, None] * contrib

    return out2d.reshape(B, S, D), loss


# revision 6
# speedup vs baseline: 1.0646x; 1.0646x over previous
"""Expert-parallel MoE (top-2 of 16 experts, SwiGLU FFN) on 8 TRN2 NeuronCores.

Contract: kernel(**inputs) takes the FULL unsharded inputs and returns the
FULL output (output tensor, load_balancing_loss), matching reference().

Strategy (expert-parallel, per the sharding hint):
  - Host: router matmul + top-2 + softmax (0.3% of FLOPs), then dispatch:
    each core owns 2 experts; its routed tokens are gathered into a packed,
    transposed, zero-padded [D, cap] activation matrix (the "all-to-all").
  - Device (SPMD x8): per expert  yT = W2^T @ (silu(W1^T xT) * (W3^T xT))
    as bf16 matmuls accumulating in f32 PSUM; weights stationary [128,128],
    tokens moving in 512-col blocks.
  - Host: combine: out[tok] += gate * yT[:, col].T  (gate folded here, so the
    device computes the unscaled expert FFN), plus the load-balancing loss.
"""

import os
import sys
import types
from contextlib import ExitStack

import ml_dtypes
import numpy as np

HIDDEN = 1024
FFN = 1024
N_EXPERTS = 16
TOP_K = 2
N_CORES = 8
P = 128

BF16 = ml_dtypes.bfloat16

_KERNEL_CACHE: dict = {}
LAST_RESULTS = None  # BassKernelResults of the most recent device run


# ---------------------------------------------------------------------------
# Optional NTFF profiling support: concourse's trace path imports
# antenv.axon_hooks, which some images lack. Register a compatible module so
# BASS_TRACE=1 profiles when the axon .so supports it and degrades otherwise.
def _ensure_axon_hooks():
    try:
        import antenv.axon_hooks  # noqa: F401
        return
    except ImportError:
        pass
    m = types.ModuleType("antenv.axon_hooks")
    m._hook = None

    def set_axon_ntff_profile_hook(h):
        m._hook = h

    def get_axon_ntff_profile_hook():
        return m._hook

    m.set_axon_ntff_profile_hook = set_axon_ntff_profile_hook
    m.get_axon_ntff_profile_hook = get_axon_ntff_profile_hook
    sys.modules["antenv.axon_hooks"] = m
    try:
        from trn_agent_boot.trn_boot import _ntff_profile_via_ctypes

        so = "/opt/axon/libaxon_pjrt.so"
        if os.path.exists(so):
            m._hook = _ntff_profile_via_ctypes(so)
    except Exception:
        pass


# ---------------------------------------------------------------------------
# Device kernel builder


def _token_blocks(cap):
    """Moving-dim blocks of <=512 columns; avoid a tail thinner than 128 by
    rebalancing the last two blocks."""
    blocks = [512] * (cap // 512)
    rem = cap % 512
    if rem:
        if rem < 128 and blocks:
            last = 512 + rem
            blocks[-1] = (last + 1) // 2
            blocks.append(last // 2)
        else:
            blocks.append(rem)
    return blocks


def _build_moe_kernel(caps, use_silu=True, d=HIDDEN, f=FFN):
    import concourse.bass as bass
    import concourse.tile as tile
    from concourse import bacc, mybir

    DT = mybir.dt.bfloat16
    PS = mybir.dt.float32
    KC = d // P
    MC = f // P
    CT = sum(caps)

    nc = bacc.Bacc("TRN2", target_bir_lowering=False, debug=False,
                   num_devices=N_CORES)

    xt = nc.dram_tensor("xt", [d, CT], DT, kind="ExternalInput")
    w1s = nc.dram_tensor("w1s", [2, d, f], DT, kind="ExternalInput")
    w2s = nc.dram_tensor("w2s", [2, f, d], DT, kind="ExternalInput")
    w3s = nc.dram_tensor("w3s", [2, d, f], DT, kind="ExternalInput")
    yt = nc.dram_tensor("yt", [d, CT], PS, kind="ExternalOutput")

    with tile.TileContext(nc) as tc, ExitStack() as ctx:
        wp = ctx.enter_context(tc.tile_pool(name="w", bufs=2))
        xp = ctx.enter_context(tc.tile_pool(name="x", bufs=3))
        hp = ctx.enter_context(tc.tile_pool(name="h", bufs=2))
        yp = ctx.enter_context(tc.tile_pool(name="y", bufs=2))
        # 8 PSUM banks: 4 x p1 + 4 x p3 during the h phase; the y phase
        # reuses the p1 slots (released once silu has consumed them).
        psA = ctx.enter_context(
            tc.tile_pool(name="psA", bufs=4, space=bass.MemorySpace.PSUM))

        QUAD = 4  # m-tiles per PSUM generation

        col = 0
        for s in range(2):
            blocks = _token_blocks(caps[s])
            # Interleave w1/w3/first-block-xt DMAs so the k-major matmuls can
            # start as soon as the first chunks land; w2 is only needed for
            # the y phase, so it loads after.
            w1t, w3t, w2t = [], [], []
            xts0 = []
            n0 = blocks[0]
            for k in range(KC):
                t = wp.tile([P, f], DT, tag=f"w1_{k}")
                nc.sync.dma_start(t[:], w1s[s, k * P:(k + 1) * P, :])
                w1t.append(t)
                t = wp.tile([P, f], DT, tag=f"w3_{k}")
                nc.sync.dma_start(t[:], w3s[s, k * P:(k + 1) * P, :])
                w3t.append(t)
                t = xp.tile([P, n0], DT, tag=f"x{k}")
                nc.sync.dma_start(t[:], xt[k * P:(k + 1) * P, col:col + n0])
                xts0.append(t)
            for m in range(MC):
                t = wp.tile([P, d], DT, tag=f"w2_{m}")
                nc.sync.dma_start(t[:], w2s[s, m * P:(m + 1) * P, :])
                w2t.append(t)

            for bi, n in enumerate(blocks):
                if bi == 0:
                    xts = xts0
                else:
                    xts = []
                    for k in range(KC):
                        t = xp.tile([P, n], DT, tag=f"x{k}")
                        nc.sync.dma_start(
                            t[:], xt[k * P:(k + 1) * P, col:col + n])
                        xts.append(t)
                hts = []
                for q in range(0, MC, QUAD):
                    nq = min(QUAD, MC - q)
                    p1s = [psA.tile([P, n], PS, tag="p1", name=f"p1_{j}")
                           for j in range(nq)]
                    p3s = [psA.tile([P, n], PS, tag="p3", name=f"p3_{j}")
                           for j in range(nq)]
                    for k in range(KC):
                        for j in range(nq):
                            m = q + j
                            nc.tensor.matmul(
                                p1s[j][:], w1t[k][:, m * P:(m + 1) * P],
                                xts[k][:], start=(k == 0), stop=(k == KC - 1))
                        for j in range(nq):
                            m = q + j
                            nc.tensor.matmul(
                                p3s[j][:], w3t[k][:, m * P:(m + 1) * P],
                                xts[k][:], start=(k == 0), stop=(k == KC - 1))
                    for j in range(nq):
                        m = q + j
                        ht = hp.tile([P, n], DT, tag=f"h{m}")
                        if use_silu:
                            nc.scalar.activation(
                                ht[:], p1s[j][:],
                                mybir.ActivationFunctionType.Silu)
                        else:
                            nc.scalar.activation(
                                ht[:], p1s[j][:],
                                mybir.ActivationFunctionType.Sigmoid)
                            nc.vector.tensor_mul(ht[:], ht[:], p1s[j][:])
                        nc.vector.tensor_mul(ht[:], ht[:], p3s[j][:])
                        hts.append(ht)
                for dd in range(KC):
                    py = psA.tile([P, n], PS, tag="p1")
                    for m in range(MC):
                        nc.tensor.matmul(
                            py[:], w2t[m][:, dd * P:(dd + 1) * P], hts[m][:],
                            start=(m == 0), stop=(m == MC - 1))
                    yo = yp.tile([P, n], PS, tag=f"y{dd % 4}")
                    nc.vector.tensor_copy(yo[:], py[:])
                    nc.sync.dma_start(yt[dd * P:(dd + 1) * P, col:col + n],
                                      yo[:])
                col += n

    nc.compile()
    return nc


def _get_kernel(caps):
    key = tuple(caps)
    if key not in _KERNEL_CACHE:
        _KERNEL_CACHE[key] = _build_moe_kernel(caps)
    return _KERNEL_CACHE[key]


# ---------------------------------------------------------------------------
# Host-side routing / dispatch / combine


def _softmax(x, axis):
    m = np.max(x, axis=axis, keepdims=True)
    e = np.exp(x - m)
    return e / np.sum(e, axis=axis, keepdims=True)


def _roundup(v, q):
    return max(q, ((int(v) + q - 1) // q) * q)


def kernel(hidden_states, router_w, w1, w2, w3):
    global LAST_RESULTS
    _ensure_axon_hooks()
    from concourse.bass_utils import run_bass_kernel_spmd

    hidden_states = np.asarray(hidden_states, dtype=np.float32)
    router_w = np.asarray(router_w, dtype=np.float32)
    w1 = np.asarray(w1, dtype=np.float32)
    w2 = np.asarray(w2, dtype=np.float32)
    w3 = np.asarray(w3, dtype=np.float32)

    B, S, D = hidden_states.shape
    T = B * S
    E = router_w.shape[1]
    x2d = hidden_states.reshape(T, D)

    # --- routing (host) ---
    logits = x2d @ router_w                                   # [T, E] f32
    order = np.argsort(-logits, axis=1, kind="stable")        # lax.top_k ties
    top_idx = order[:, :TOP_K]                                # [T, K]
    top_vals = np.take_along_axis(logits, top_idx, axis=1)
    gates = _softmax(top_vals, axis=1).astype(np.float32)     # [T, K]

    probs = _softmax(logits, axis=1)
    usage = probs.mean(axis=0, dtype=np.float32)
    loss = np.float32(E) * np.sum(usage * usage, dtype=np.float32)
    loss = np.float32(loss)

    # --- dispatch: group (token, gate) pairs by expert ---
    flat_expert = top_idx.ravel()
    flat_tok = np.repeat(np.arange(T, dtype=np.int64), TOP_K)
    flat_gate = gates.ravel()
    grp = np.argsort(flat_expert, kind="stable")
    counts = np.bincount(flat_expert, minlength=E)
    starts = np.zeros(E + 1, np.int64)
    np.cumsum(counts, out=starts[1:])
    tok_of = [flat_tok[grp[starts[e]:starts[e + 1]]] for e in range(E)]
    gate_of = [flat_gate[grp[starts[e]:starts[e + 1]]] for e in range(E)]

    # assign experts to (core, slot): slot0 = 8 busiest, slot1 = 8 smallest,
    # paired big-with-small so per-slot capacity (a max over cores) is tight
    by_load = np.argsort(-counts, kind="stable")
    slot_experts = [[int(by_load[c]), int(by_load[15 - c])]
                    for c in range(N_CORES)]
    cap0 = max(int(counts[es[0]]) for es in slot_experts)
    cap1 = max(int(counts[es[1]]) for es in slot_experts)
    # keep a floor so degenerate routings still build a valid kernel
    caps = (max(cap0, 128), max(cap1, 128))
    cap0, cap1 = caps
    CT = cap0 + cap1

    xbf = x2d.astype(BF16)
    w1bf = w1.astype(BF16)
    w2bf = w2.astype(BF16)
    w3bf = w3.astype(BF16)

    in_maps = []
    for c in range(N_CORES):
        e0, e1 = slot_experts[c]
        xt = np.zeros((D, CT), BF16)
        xt[:, :len(tok_of[e0])] = xbf[tok_of[e0]].T
        xt[:, cap0:cap0 + len(tok_of[e1])] = xbf[tok_of[e1]].T
        in_maps.append({
            "xt": xt,
            "w1s": np.stack([w1bf[e0], w1bf[e1]]),
            "w2s": np.stack([w2bf[e0], w2bf[e1]]),
            "w3s": np.stack([w3bf[e0], w3bf[e1]]),
        })

    nc = _get_kernel(caps)
    res = run_bass_kernel_spmd(nc, in_maps, list(range(N_CORES)))
    LAST_RESULTS = res

    # --- combine (host): out[tok] += gate * yT[:, col]  ---
    out2d = np.zeros((T, D), np.float32)
    for c in range(N_CORES):
        ytc = res.results[c]["yt"]                            # [D, CT] f32
        for s, off in ((0, 0), (1, cap0)):
            e = slot_experts[c][s]
            n = len(tok_of[e])
            if n == 0:
                continue
            contrib = ytc[:, off:off + n].T                   # [n, D]
            out2d[tok_of[e]] += gate_of[e][:, None] * contrib

    return out2d.reshape(B, S, D), loss
